# revision 20
# baseline (speedup 1.0000x reference)
"""Trainium2 Bass kernel for nn_PitchLoss (segment_reduce).

Math: for each note k with frame range [a_k, b_k), the reference builds a
dense (T, N) mask and computes per-note means of gen_f0 / t_f0 over the
range, then loss = mean((|mean_gen - mean_ref| > 0.5)).

Since each note is a contiguous frame range, per-note sums are prefix-sum
differences: with d = gen_f0 - t_f0 and cse[x] = sum(d[0:x]),
    |mean_gen_k - mean_ref_k| = |cse[b_k] - cse[a_k]| / (b_k - a_k)
so  verdict_k = (b_k > a_k) & (|cse[b_k] - cse[a_k]| > 0.5 * (b_k - a_k))
which also reproduces the reference's empty-segment NaN > 0.5 == False.

Sharding: notes across 8 cores (128 notes/core); gen_f0/t_f0 replicated.

Per core, with d laid out (128, 256) and scz = per-row inclusive scan
(col 0 = 0, col 256 = row sum), split x = 256r + c (c = x & 255):
    cse[x] = SUM_q [x >= 256(q+1)] * rowsum[q]              (W2 gather)
           + SUM_p [256p <= x < 256p+256] * scz[p, c]       (onep gather)
Both gathers are one-hot matmuls; the onep gather's in-row column select
uses a DVE one-hot multiply + accumulator, and the W2 gather runs with
difference weights w2d = W2_b - W2_a in {-1,0,1} so one matmul yields
d2 = roffsum[b] - roffsum[a] directly.  x == T falls out naturally: the
onep column is all-zero (contributes 0) and W2 sums every row.
delta = (val_b - val_a) + d2; the verdict count is reduced on-device to
ONE scalar via a final pos^T @ cmp matmul; the host sums 8 counts ->
loss (/1024 is a pow2, so the host mean is exact).

Layout: the host packs one (128, 770) fp32 tensor per core, each row =
[gen (256) | ref (256) | the 256 onset/offset ints bit-cast to f32,
replicated per row | this row's own (onset, offset) pair].  Each DMA
queue loads its 64 rows as ONE DMA with one 3080-byte descriptor per
row.  DMA cost here is descriptor-count-bound (~100ns+ per descriptor
aggregate): splitting the load into per-slice DMAs (6x64x1KB) measured
~2.5us SLOWER than 128x3KB, same-address broadcast reads ran at
~60GB/s, and gpsimd partition_broadcast pulls in a different ucode
library (~8us MODIFY_POOL_CONFIG load).  Replicating the indices per
row costs 128KB of extra wire (~0.4us) and is the cheapest option.

Perf notes vs the first working version (28.3us):
 - Output is 1 fp32 (one DMA descriptor).  The old (128,1) verdict DMA
   needed 128 4-byte descriptors; their completion increments trickled
   in over ~6us (descriptor processing dominates tiny SBUF-source DMAs).
   The output DMA is also fired once EARLY (same queue, stale value,
   overwritten in order) so the ring is awake when the real fire lands.
 - All matmuls run bf16 single-pass with an hi/lo split of the fp32 scan
   (hi = bf16(x), lo = bf16(x - hi), accumulated in the same PSUM bank).
   One-hot weights are exact in bf16; |delta| error <= ~1e-3 against a
   >= 0.2 decision margin on this input.  An fp32 matmul costs
   2x(LDWEIGHTS+MATMUL) passes (~2.1us for 128x257); bf16 pairs ~0.7us.
 - hi/lo splits use TENSOR_SCALAR/STT forms (~350-420ns) instead of
   CAST/TENSOR_TENSOR (~610-710ns for 257 cols).
 - ge/ge2 run before the scan so gpsimd's ~1us tensor_tensor combine
   (onep = ge - ge2) overlaps the scan and the hi/lo split.
 - No Activation-engine compute: an ACTIVATE pulls a ~1.3us
   ACT_TABLE_LOAD into the scalar engine's preamble, delaying its DMA
   dispatch.  The final PSUM->SBUF copy runs on DVE instead.
 - Raw Bacc engine programs with hand-placed semaphores (no TileContext -
   its entry/exit barrier costs ~15us on a ~5us kernel).  Engine-order
   hazards: DVE reads racing the immediately preceding op's writeback are
   padded with real ops (interleaved independent chains; memset does NOT
   count); gpsimd ucode cores overlap, so every gpsimd op incs s_g and
   consumers wait on cumulative counts.
"""

from contextlib import ExitStack

import numpy as np

import concourse.bacc as bacc
import concourse.bass as bass
from concourse import mybir
from concourse.bass_utils import run_bass_kernel_spmd

T = 32768           # frames
N = 1024            # notes
NCORES = 8
NPC = N // NCORES   # notes per core
P = 128             # partitions
F = T // P          # 256 frames per partition row
FP1 = F + 1         # 257: scan columns (col 256 = row sum)
K2 = 2 * NPC        # 256: onsets ++ offsets
W = 3 * F + 2       # 770 packed row floats [gen|ref|idx|oc pair]
PLO = 64            # row split between the two DMA queues
DT = mybir.dt.float32
BF = mybir.dt.bfloat16
I32 = mybir.dt.int32
ALU = mybir.AluOpType


def build_nc(debug_outs=False):
    # detect_race_conditions=False: the CoreSim race detector does not credit
    # same-engine program order, but HW engines execute their queues in order
    # (DVE drains its pipe after every op); gpsimd, whose ucode cores do
    # overlap, is synchronized explicitly below.
    nc = bacc.Bacc("TRN2", target_bir_lowering=False, debug=False,
                   detect_race_conditions=False)
    f0x = nc.dram_tensor("f0x", [P, W], DT, kind="ExternalInput")
    onoff = nc.dram_tensor("onoff", [2 * K2], I32, kind="ExternalInput")
    out = nc.dram_tensor("verdict", [1], DT, kind="ExternalOutput")
    dbg = {}
    if debug_outs:
        for name, shape, dt in [
                ("dbg_scz", [P, FP1], DT), ("dbg_xb", [P, K2], DT),
                ("dbg_onep", [P, K2], BF), ("dbg_w2d", [P, NPC], BF),
                ("dbg_onefa", [P, F], DT), ("dbg_onefb", [P, F], DT),
                ("dbg_fcf", [P, 2], DT), ("dbg_val", [P, 2], DT),
                ("dbg_szhi", [P, FP1], BF), ("dbg_szlo", [P, FP1], BF),
                ("dbg_msum", [P, 1], DT), ("dbg_halfm", [P, 1], DT),
                ("dbg_d2", [P, 1], DT),
                ("dbg_delta", [P, 1], DT), ("dbg_absd", [P, 1], DT),
                ("dbg_cmp", [P, 1], BF), ("dbg_pos", [P, 1], BF),
                ("dbg_rga", [P, F], DT), ("dbg_rgb", [P, F], DT)]:
            dbg[name] = nc.dram_tensor(name, shape, dt, kind="ExternalOutput")

    with ExitStack() as ctx:
        def sb(name, shape, dt=DT):
            return ctx.enter_context(nc.sbuf_tensor(name, shape, dt))

        def pst(name, shape):
            return ctx.enter_context(nc.psum_tensor(name, shape, DT))

        # constants
        p256 = sb("p256", [P, 1])          # 256p
        p256e = sb("p256e", [P, 1])        # 256p + 256
        iota_f = sb("iota_f", [P, F])      # 0..255 per row
        # data tiles
        frt = sb("frt", [P, W])            # [gen|ref|idx|oc]
        xb = sb("xb", [P, K2])
        fci = sb("fci", [P, 2], I32)
        xf = sb("xf", [P, 2])
        fcf = sb("fcf", [P, 2])
        ge = sb("ge", [P, K2])
        ge2 = sb("ge2", [P, K2])
        onep = sb("onep", [P, K2], BF)
        w2d = sb("w2d", [P, NPC], BF)
        msum = sb("msum", [P, 1])
        halfm = sb("halfm", [P, 1])
        posb = sb("posb", [P, 1], BF)
        scr0 = sb("scr0", [P, 1])          # spacer scratch
        scz = sb("scz", [P, FP1])
        scz_hi = sb("scz_hi", [P, FP1], BF)
        scz_lo = sb("scz_lo", [P, FP1], BF)
        onef_a = sb("onef_a", [P, F])
        onef_b = sb("onef_b", [P, F])
        scr = sb("scr", [P, F])
        val = sb("val", [P, 2])
        delta = sb("delta", [P, 1])
        absd = sb("absd", [P, 1])
        cmpb = sb("cmpb", [P, 1], BF)
        vs_s = sb("vs_s", [1, 1])
        if debug_outs:
            rga_cp = sb("rga_cp", [P, F])
            rgb_cp = sb("rgb_cp", [P, F])
            d2_cp = sb("d2_cp", [P, 1])
        # psum (distinct banks)
        rga_ps = pst("rga_ps", [P, F])
        rgb_ps = pst("rgb_ps", [P, F])
        d2_ps = pst("d2_ps", [P, 1])
        vs_ps = pst("vs_ps", [1, 1])

        s_fr = ctx.enter_context(nc.semaphore("s_fr"))
        s_fr2 = ctx.enter_context(nc.semaphore("s_fr2"))
        s_g = ctx.enter_context(nc.semaphore("s_g"))
        s_v = ctx.enter_context(nc.semaphore("s_v"))
        s_t = ctx.enter_context(nc.semaphore("s_t"))
        s_out = ctx.enter_context(nc.semaphore("s_out"))
        block = ctx.enter_context(nc.Block())

        @block.sync
        def _(sync):
            sync.dma_start(out=frt[0:PLO, :],
                           in_=f0x[0:PLO, :]).then_inc(s_fr, 16)
            # early fire of the output DMA: same queue, stale vs_s value,
            # overwritten in order by the real fire below; wakes the idle
            # ring (~0.8us) while the verdict tail finishes.
            sync.wait_ge(s_v, 4)
            sync.dma_start(out=out[:].rearrange("(p f) -> p f", f=1),
                           in_=vs_s[0:1, 0:1]).then_inc(s_out, 16)
            sync.wait_ge(s_v, 5)
            sync.dma_start(out=out[:].rearrange("(p f) -> p f", f=1),
                           in_=vs_s[0:1, 0:1]).then_inc(s_out, 16)
            n_out = 32
            if debug_outs:
                sync.wait_ge(s_v, 6)
                sync.wait_ge(s_g, 5)
                for name, tile in [
                        ("dbg_scz", scz), ("dbg_xb", xb),
                        ("dbg_onep", onep), ("dbg_w2d", w2d),
                        ("dbg_onefa", onef_a), ("dbg_onefb", onef_b),
                        ("dbg_fcf", fcf), ("dbg_val", val),
                        ("dbg_szhi", scz_hi), ("dbg_szlo", scz_lo),
                        ("dbg_msum", msum), ("dbg_halfm", halfm),
                        ("dbg_d2", d2_cp),
                        ("dbg_delta", delta), ("dbg_absd", absd),
                        ("dbg_cmp", cmpb), ("dbg_pos", posb),
                        ("dbg_rga", rga_cp), ("dbg_rgb", rgb_cp)]:
                    sync.dma_start(out=dbg[name][:], in_=tile[:]) \
                        .then_inc(s_out, 16)
                    n_out += 16
            sync.wait_ge(s_out, n_out)

        @block.scalar
        def _(act):
            act.dma_start(out=frt[PLO:P, :],
                          in_=f0x[PLO:P, :]).then_inc(s_fr2, 16)

        @block.gpsimd
        def _(gpsimd):
            # gpsimd ucode cores overlap: every op incs s_g; consumers of a
            # gpsimd result wait on the cumulative count (all earlier-issued
            # ops complete by then, since each op incs exactly once).
            gp = nc.gpsimd
            gpsimd.iota(p256[:], pattern=[[0, 1]], base=0,
                        channel_multiplier=F,
                        allow_small_or_imprecise_dtypes=True).then_inc(s_g, 1)
            gpsimd.iota(p256e[:], pattern=[[0, 1]], base=F,
                        channel_multiplier=F,
                        allow_small_or_imprecise_dtypes=True).then_inc(s_g, 1)
            gpsimd.iota(iota_f[:], pattern=[[1, F]], base=0,
                        channel_multiplier=0,
                        allow_small_or_imprecise_dtypes=True).then_inc(s_g, 1)
            # one-hot matmul weights from the DVE compares:
            # onep[p,k] = [256p <= x_k < 256p+256] = ge - ge2 (0/1, bf16-exact)
            # w2d[p,k]  = [b_k >= 256(p+1)] - [a_k >= 256(p+1)]  in {-1,0,1}
            gpsimd.wait_ge(s_v, 1)     # ge + ge2
            gp.tensor_tensor(onep[:], ge[:], ge2[:],
                             ALU.subtract).then_inc(s_g, 1)          # 4
            gp.tensor_tensor(w2d[:], ge2[:, NPC:K2], ge2[:, 0:NPC],
                             ALU.subtract).then_inc(s_g, 1)          # 5

        @block.tensor
        def _(tensor):
            # all matmuls bf16 single-pass; fp32 scan data enters as hi/lo
            # bf16 pairs accumulated in the same PSUM bank.
            tensor.wait_ge(s_g, 4)     # onep
            tensor.wait_ge(s_v, 2)     # scz_hi
            nc.tensor.matmul(rga_ps[:], onep[:, 0:NPC], scz_hi[:, 0:F],
                             start=True, stop=False)
            tensor.wait_ge(s_v, 3)     # scz_lo
            nc.tensor.matmul(rga_ps[:], onep[:, 0:NPC], scz_lo[:, 0:F],
                             start=False, stop=True).then_inc(s_t, 1)
            nc.tensor.matmul(rgb_ps[:], onep[:, NPC:K2], scz_hi[:, 0:F],
                             start=True, stop=False)
            nc.tensor.matmul(rgb_ps[:], onep[:, NPC:K2], scz_lo[:, 0:F],
                             start=False, stop=True).then_inc(s_t, 1)
            tensor.wait_ge(s_g, 5)     # w2d
            nc.tensor.matmul(d2_ps[:], w2d[:], scz_hi[:, F:FP1],
                             start=True, stop=False)
            nc.tensor.matmul(d2_ps[:], w2d[:], scz_lo[:, F:FP1],
                             start=False, stop=True).then_inc(s_t, 1)
            tensor.wait_ge(s_v, 4)     # cmpb (posb is earlier in v-order)
            nc.tensor.matmul(vs_ps[0:1, 0:1], posb[:], cmpb[:],
                             start=True, stop=True).then_inc(s_t, 1)

        @block.vector
        def _(vector):
            vec = nc.vector
            vec.memset(scz[:, 0:1], 0.0)
            # DVE gap-0 RAW hazards get >=1 real op between each dependent
            # pair (interleaved independent chains; memset does NOT count).
            vector.wait_ge(s_fr, 16)
            vector.wait_ge(s_fr2, 16)
            vec.tensor_copy(xb[:], frt[:, 2 * F:3 * F].bitcast(I32))
            vec.tensor_scalar(fci[:], frt[:, 3 * F:W].bitcast(I32), 255,
                              None, op0=ALU.bitwise_and)
            vector.wait_ge(s_g, 2)
            vec.tensor_scalar(ge[:], xb[:], p256[:], None, op0=ALU.is_ge)
            vec.tensor_scalar(ge2[:], xb[:], p256e[:], None,
                              op0=ALU.is_ge).then_inc(s_v, 1)        # 1
            # fused diff + inclusive scan: state = (gen + state) - ref
            vec.tensor_tensor_scan(scz[:, 1:FP1], frt[:, 0:F],
                                   frt[:, F:2 * F], 0.0,
                                   op0=ALU.add, op1=ALU.subtract)
            vec.tensor_copy(xf[:], frt[:, 3 * F:W].bitcast(I32))
            vec.tensor_scalar(scz_hi[:], scz[:], 0.0, None,
                              op0=ALU.add).then_inc(s_v, 1)          # 2
            vec.tensor_copy(fcf[:], fci[:])
            vec.scalar_tensor_tensor(scz_lo[:], scz[:], 0.0, scz_hi[:],
                                     op0=ALU.add, op1=ALU.subtract) \
               .then_inc(s_v, 1)                                     # 3
            vec.tensor_sub(msum[:], xf[:, 1:2], xf[:, 0:1])
            vector.wait_ge(s_g, 3)     # iota_f
            vec.tensor_scalar(onef_a[:], iota_f[:], fcf[:, 0:1], None,
                              op0=ALU.is_equal)
            vec.tensor_scalar(onef_b[:], iota_f[:], fcf[:, 1:2], None,
                              op0=ALU.is_equal)
            # gather tails: select col c of the gathered row via one-hot
            # multiply + DVE accumulator
            vector.wait_ge(s_t, 1)     # rga
            vec.scalar_tensor_tensor(scr[:], rga_ps[:], 1.0, onef_a[:],
                                     op0=ALU.mult, op1=ALU.mult,
                                     accum_out=val[:, 0:1])
            vector.wait_ge(s_t, 2)     # rgb
            vec.scalar_tensor_tensor(scr[:], rgb_ps[:], 1.0, onef_b[:],
                                     op0=ALU.mult, op1=ALU.mult,
                                     accum_out=val[:, 1:2])
            vec.tensor_scalar(halfm[:], msum[:], 0.5, None, op0=ALU.mult)
            vector.wait_ge(s_t, 3)     # d2
            vec.scalar_tensor_tensor(delta[:], val[:, 1:2], val[:, 0:1],
                                     d2_ps[:], op0=ALU.subtract,
                                     op1=ALU.add)
            vec.tensor_scalar(posb[:], msum[:], 0.0, None, op0=ALU.is_gt)
            vec.scalar_tensor_tensor(absd[:], delta[:], -1.0, delta[:],
                                     op0=ALU.mult, op1=ALU.max)
            vec.tensor_scalar(scr0[:], msum[:], 4.0, None, op0=ALU.mult)
            vec.tensor_tensor(cmpb[:], halfm[:], absd[:],
                              ALU.is_lt).then_inc(s_v, 1)            # 4
            vector.wait_ge(s_t, 4)     # verdict count in PSUM
            vec.tensor_copy(vs_s[0:1, 0:1], vs_ps[0:1, 0:1]) \
               .then_inc(s_v, 1)                                     # 5
            if debug_outs:
                vec.tensor_copy(rga_cp[:], rga_ps[:])
                vec.tensor_copy(rgb_cp[:], rgb_ps[:])
                vec.tensor_copy(d2_cp[:], d2_ps[:]).then_inc(s_v, 1)  # 6

    nc.finalize()
    return nc


_NC_CACHE = {}


def _get_nc(debug_outs=False):
    if debug_outs not in _NC_CACHE:
        _NC_CACHE[debug_outs] = build_nc(debug_outs)
    return _NC_CACHE[debug_outs]


def _pack_onoff(on, off):
    # [pairs (on_p, off_p) x128 | on x128 | off x128]
    pairs = np.stack([on, off], axis=1).ravel()
    return np.concatenate([pairs, on, off])


def _pack_f0x(gen, ref, on, off):
    # per row: [gen | ref | onset++offset ints bitcast to f32, replicated
    # per row | this row's own (onset, offset) pair]
    g = gen.reshape(P, F)
    r = ref.reshape(P, F)
    idx = np.concatenate([on, off]).astype(np.int32).view(np.float32)
    pair = np.stack([on, off], axis=1).astype(np.int32).view(np.float32)
    return np.concatenate(
        [g, r, np.broadcast_to(idx, (P, K2)), pair], axis=1).copy()


def _run(inputs, debug_outs=False, **kwargs):
    gen = np.ascontiguousarray(inputs["gen_f0"], dtype=np.float32)
    ref = np.ascontiguousarray(inputs["t_f0"], dtype=np.float32)
    on = np.ascontiguousarray(inputs["onset_times"], dtype=np.int32)
    off = np.ascontiguousarray(inputs["offset_times"], dtype=np.int32)

    nc = _get_nc(debug_outs)
    in_maps = [
        {
            "f0x": _pack_f0x(gen, ref, on[c * NPC:(c + 1) * NPC],
                             off[c * NPC:(c + 1) * NPC]),
            "onoff": _pack_onoff(on[c * NPC:(c + 1) * NPC],
                                 off[c * NPC:(c + 1) * NPC]),
        }
        for c in range(NCORES)
    ]
    return run_bass_kernel_spmd(nc, in_maps, core_ids=list(range(NCORES)),
                                **kwargs)


def kernel(**inputs):
    res = _run(inputs)
    counts = np.stack([res.results[c]["verdict"] for c in range(NCORES)])
    return np.asarray(counts.sum() / np.float32(N), dtype=np.float32)


# revision 22
# speedup vs baseline: 1.1526x; 1.1526x over previous
"""Trainium2 Bass kernel for nn_PitchLoss (segment_reduce).

Math: for each note k with frame range [a_k, b_k), the reference builds a
dense (T, N) mask and computes per-note means of gen_f0 / t_f0 over the
range, then loss = mean((|mean_gen - mean_ref| > 0.5)).

Since each note is a contiguous frame range, per-note sums are prefix-sum
differences: with d = gen_f0 - t_f0 and cse[x] = sum(d[0:x]),
    |mean_gen_k - mean_ref_k| = |cse[b_k] - cse[a_k]| / (b_k - a_k)
so  verdict_k = (b_k > a_k) & (|cse[b_k] - cse[a_k]| > 0.5 * (b_k - a_k))
which also reproduces the reference's empty-segment NaN > 0.5 == False.

Sharding: notes across 8 cores (128 notes/core); gen_f0/t_f0 replicated.

Per core, with d laid out (128, 256) and scz = per-row inclusive scan
(col 0 = 0, col 256 = row sum), split x = 256r + c (c = x & 255):
    cse[x] = SUM_q [x >= 256(q+1)] * rowsum[q]              (W2 gather)
           + SUM_p [256p <= x < 256p+256] * scz[p, c]       (onep gather)
Both gathers are one-hot matmuls; the onep gather's in-row column select
uses a DVE one-hot multiply + accumulator, and the W2 gather runs with
difference weights w2d = W2_b - W2_a in {-1,0,1} so one matmul yields
d2 = roffsum[b] - roffsum[a] directly.  x == T falls out naturally: the
onep column is all-zero (contributes 0) and W2 sums every row.
delta = (val_b - val_a) + d2; the verdict count is reduced on-device to
ONE scalar via a final pos^T @ cmp matmul; the host sums 8 counts ->
loss (/1024 is a pow2, so the host mean is exact).

Layout: the host packs one (128, 770) fp32 tensor per core, each row =
[gen (256) | ref (256) | the 256 onset/offset ints bit-cast to f32,
replicated per row | this row's own (onset, offset) pair].  Each DMA
queue loads its 64 rows as ONE DMA with one 3080-byte descriptor per
row.  DMA cost here is descriptor-count-bound (~100ns+ per descriptor
aggregate): splitting the load into per-slice DMAs (6x64x1KB) measured
~2.5us SLOWER than 128x3KB, same-address broadcast reads ran at
~60GB/s, and gpsimd partition_broadcast pulls in a different ucode
library (~8us MODIFY_POOL_CONFIG load).  Replicating the indices per
row costs 128KB of extra wire (~0.4us) and is the cheapest option.

Perf notes vs the first working version (28.3us):
 - Output is 1 fp32 (one DMA descriptor).  The old (128,1) verdict DMA
   needed 128 4-byte descriptors; their completion increments trickled
   in over ~6us (descriptor processing dominates tiny SBUF-source DMAs).
   (An extra early "prewarm" fire of this DMA was tried and measured
   NET-NEGATIVE: the second dispatch serializes behind it on the queue
   engine, costing more than the ~0.4us ring-wakeup it saves.)
 - All matmuls run bf16 single-pass with an hi/lo split of the fp32 scan
   (hi = bf16(x), lo = bf16(x - hi), accumulated in the same PSUM bank).
   One-hot weights are exact in bf16; |delta| error <= ~1e-3 against a
   >= 0.2 decision margin on this input.  An fp32 matmul costs
   2x(LDWEIGHTS+MATMUL) passes (~2.1us for 128x257); bf16 pairs ~0.7us.
 - hi/lo splits use TENSOR_SCALAR/STT forms (~350-420ns) instead of
   CAST/TENSOR_TENSOR (~610-710ns for 257 cols).
 - ge/ge2 run before the scan so gpsimd's ~1us tensor_tensor combine
   (onep = ge - ge2) overlaps the scan and the hi/lo split.
 - No Activation-engine compute: an ACTIVATE pulls a ~1.3us
   ACT_TABLE_LOAD into the scalar engine's preamble, delaying its DMA
   dispatch.  The final PSUM->SBUF copy runs on DVE instead.
 - Raw Bacc engine programs with hand-placed semaphores (no TileContext -
   its entry/exit barrier costs ~15us on a ~5us kernel).  Engine-order
   hazards: DVE reads racing the immediately preceding op's writeback are
   padded with real ops (interleaved independent chains; memset does NOT
   count); gpsimd ucode cores overlap, so every gpsimd op incs s_g and
   consumers wait on cumulative counts.
"""

from contextlib import ExitStack

import numpy as np

import concourse.bacc as bacc
import concourse.bass as bass
from concourse import mybir
from concourse.bass_utils import run_bass_kernel_spmd

T = 32768           # frames
N = 1024            # notes
NCORES = 8
NPC = N // NCORES   # notes per core
P = 128             # partitions
F = T // P          # 256 frames per partition row
FP1 = F + 1         # 257: scan columns (col 256 = row sum)
K2 = 2 * NPC        # 256: onsets ++ offsets
W = 3 * F + 2       # 770 packed row floats [gen|ref|idx|oc pair]
PLO = 64            # row split between the two DMA queues
DT = mybir.dt.float32
BF = mybir.dt.bfloat16
I32 = mybir.dt.int32
ALU = mybir.AluOpType


def build_nc(debug_outs=False):
    # detect_race_conditions=False: the CoreSim race detector does not credit
    # same-engine program order, but HW engines execute their queues in order
    # (DVE drains its pipe after every op); gpsimd, whose ucode cores do
    # overlap, is synchronized explicitly below.
    nc = bacc.Bacc("TRN2", target_bir_lowering=False, debug=False,
                   detect_race_conditions=False)
    f0x = nc.dram_tensor("f0x", [P, W], DT, kind="ExternalInput")
    onoff = nc.dram_tensor("onoff", [2 * K2], I32, kind="ExternalInput")
    out = nc.dram_tensor("verdict", [1], DT, kind="ExternalOutput")
    dbg = {}
    if debug_outs:
        for name, shape, dt in [
                ("dbg_scz", [P, FP1], DT), ("dbg_xb", [P, K2], DT),
                ("dbg_onep", [P, K2], BF), ("dbg_w2d", [P, NPC], BF),
                ("dbg_onefa", [P, F], DT), ("dbg_onefb", [P, F], DT),
                ("dbg_fcf", [P, 2], DT), ("dbg_val", [P, 2], DT),
                ("dbg_szhi", [P, FP1], BF), ("dbg_szlo", [P, FP1], BF),
                ("dbg_msum", [P, 1], DT), ("dbg_halfm", [P, 1], DT),
                ("dbg_d2", [P, 1], DT),
                ("dbg_delta", [P, 1], DT), ("dbg_absd", [P, 1], DT),
                ("dbg_cmp", [P, 1], BF), ("dbg_pos", [P, 1], BF),
                ("dbg_rga", [P, F], DT), ("dbg_rgb", [P, F], DT)]:
            dbg[name] = nc.dram_tensor(name, shape, dt, kind="ExternalOutput")

    with ExitStack() as ctx:
        def sb(name, shape, dt=DT):
            return ctx.enter_context(nc.sbuf_tensor(name, shape, dt))

        def pst(name, shape):
            return ctx.enter_context(nc.psum_tensor(name, shape, DT))

        # constants
        p256 = sb("p256", [P, 1])          # 256p
        p256e = sb("p256e", [P, 1])        # 256p + 256
        iota_f = sb("iota_f", [P, F])      # 0..255 per row
        # data tiles
        frt = sb("frt", [P, W])            # [gen|ref|idx|oc]
        xb = sb("xb", [P, K2])
        fci = sb("fci", [P, 2], I32)
        xf = sb("xf", [P, 2])
        fcf = sb("fcf", [P, 2])
        ge = sb("ge", [P, K2])
        ge2 = sb("ge2", [P, K2])
        onep = sb("onep", [P, K2], BF)
        w2d = sb("w2d", [P, NPC], BF)
        msum = sb("msum", [P, 1])
        halfm = sb("halfm", [P, 1])
        posb = sb("posb", [P, 1], BF)
        scr0 = sb("scr0", [P, 1])          # spacer scratch
        scz = sb("scz", [P, FP1])
        scz_hi = sb("scz_hi", [P, FP1], BF)
        scz_lo = sb("scz_lo", [P, FP1], BF)
        onef_a = sb("onef_a", [P, F])
        onef_b = sb("onef_b", [P, F])
        scr = sb("scr", [P, F])
        val = sb("val", [P, 2])
        delta = sb("delta", [P, 1])
        absd = sb("absd", [P, 1])
        cmpb = sb("cmpb", [P, 1], BF)
        vs_s = sb("vs_s", [1, 1])
        if debug_outs:
            rga_cp = sb("rga_cp", [P, F])
            rgb_cp = sb("rgb_cp", [P, F])
            d2_cp = sb("d2_cp", [P, 1])
        # psum (distinct banks)
        rga_ps = pst("rga_ps", [P, F])
        rgb_ps = pst("rgb_ps", [P, F])
        d2_ps = pst("d2_ps", [P, 1])
        vs_ps = pst("vs_ps", [1, 1])

        s_fr = ctx.enter_context(nc.semaphore("s_fr"))
        s_fr2 = ctx.enter_context(nc.semaphore("s_fr2"))
        s_g = ctx.enter_context(nc.semaphore("s_g"))
        s_v = ctx.enter_context(nc.semaphore("s_v"))
        s_t = ctx.enter_context(nc.semaphore("s_t"))
        s_out = ctx.enter_context(nc.semaphore("s_out"))
        block = ctx.enter_context(nc.Block())

        @block.sync
        def _(sync):
            sync.dma_start(out=frt[0:PLO, :],
                           in_=f0x[0:PLO, :]).then_inc(s_fr, 16)
            sync.wait_ge(s_v, 5)
            sync.dma_start(out=out[:].rearrange("(p f) -> p f", f=1),
                           in_=vs_s[0:1, 0:1]).then_inc(s_out, 16)
            n_out = 16
            if debug_outs:
                sync.wait_ge(s_v, 6)
                sync.wait_ge(s_g, 5)
                for name, tile in [
                        ("dbg_scz", scz), ("dbg_xb", xb),
                        ("dbg_onep", onep), ("dbg_w2d", w2d),
                        ("dbg_onefa", onef_a), ("dbg_onefb", onef_b),
                        ("dbg_fcf", fcf), ("dbg_val", val),
                        ("dbg_szhi", scz_hi), ("dbg_szlo", scz_lo),
                        ("dbg_msum", msum), ("dbg_halfm", halfm),
                        ("dbg_d2", d2_cp),
                        ("dbg_delta", delta), ("dbg_absd", absd),
                        ("dbg_cmp", cmpb), ("dbg_pos", posb),
                        ("dbg_rga", rga_cp), ("dbg_rgb", rgb_cp)]:
                    sync.dma_start(out=dbg[name][:], in_=tile[:]) \
                        .then_inc(s_out, 16)
                    n_out += 16
            sync.wait_ge(s_out, n_out)

        @block.scalar
        def _(act):
            act.dma_start(out=frt[PLO:P, :],
                          in_=f0x[PLO:P, :]).then_inc(s_fr2, 16)

        @block.gpsimd
        def _(gpsimd):
            # gpsimd ucode cores overlap: every op incs s_g; consumers of a
            # gpsimd result wait on the cumulative count (all earlier-issued
            # ops complete by then, since each op incs exactly once).
            gp = nc.gpsimd
            gpsimd.iota(p256[:], pattern=[[0, 1]], base=0,
                        channel_multiplier=F,
                        allow_small_or_imprecise_dtypes=True).then_inc(s_g, 1)
            gpsimd.iota(p256e[:], pattern=[[0, 1]], base=F,
                        channel_multiplier=F,
                        allow_small_or_imprecise_dtypes=True).then_inc(s_g, 1)
            gpsimd.iota(iota_f[:], pattern=[[1, F]], base=0,
                        channel_multiplier=0,
                        allow_small_or_imprecise_dtypes=True).then_inc(s_g, 1)
            # one-hot matmul weights from the DVE compares:
            # onep[p,k] = [256p <= x_k < 256p+256] = ge - ge2 (0/1, bf16-exact)
            # w2d[p,k]  = [b_k >= 256(p+1)] - [a_k >= 256(p+1)]  in {-1,0,1}
            gpsimd.wait_ge(s_v, 1)     # ge + ge2
            gp.tensor_tensor(onep[:], ge[:], ge2[:],
                             ALU.subtract).then_inc(s_g, 1)          # 4
            gp.tensor_tensor(w2d[:], ge2[:, NPC:K2], ge2[:, 0:NPC],
                             ALU.subtract).then_inc(s_g, 1)          # 5

        @block.tensor
        def _(tensor):
            # all matmuls bf16 single-pass; fp32 scan data enters as hi/lo
            # bf16 pairs accumulated in the same PSUM bank.
            tensor.wait_ge(s_g, 4)     # onep
            tensor.wait_ge(s_v, 2)     # scz_hi
            nc.tensor.matmul(rga_ps[:], onep[:, 0:NPC], scz_hi[:, 0:F],
                             start=True, stop=False)
            tensor.wait_ge(s_v, 3)     # scz_lo
            nc.tensor.matmul(rga_ps[:], onep[:, 0:NPC], scz_lo[:, 0:F],
                             start=False, stop=True).then_inc(s_t, 1)
            nc.tensor.matmul(rgb_ps[:], onep[:, NPC:K2], scz_hi[:, 0:F],
                             start=True, stop=False)
            nc.tensor.matmul(rgb_ps[:], onep[:, NPC:K2], scz_lo[:, 0:F],
                             start=False, stop=True).then_inc(s_t, 1)
            tensor.wait_ge(s_g, 5)     # w2d
            nc.tensor.matmul(d2_ps[:], w2d[:], scz_hi[:, F:FP1],
                             start=True, stop=False)
            nc.tensor.matmul(d2_ps[:], w2d[:], scz_lo[:, F:FP1],
                             start=False, stop=True).then_inc(s_t, 1)
            tensor.wait_ge(s_v, 4)     # cmpb (posb is earlier in v-order)
            nc.tensor.matmul(vs_ps[0:1, 0:1], posb[:], cmpb[:],
                             start=True, stop=True).then_inc(s_t, 1)

        @block.vector
        def _(vector):
            vec = nc.vector
            vec.memset(scz[:, 0:1], 0.0)
            # DVE gap-0 RAW hazards get >=1 real op between each dependent
            # pair (interleaved independent chains; memset does NOT count).
            vector.wait_ge(s_fr, 16)
            vector.wait_ge(s_fr2, 16)
            vec.tensor_copy(xb[:], frt[:, 2 * F:3 * F].bitcast(I32))
            vec.tensor_scalar(fci[:], frt[:, 3 * F:W].bitcast(I32), 255,
                              None, op0=ALU.bitwise_and)
            vector.wait_ge(s_g, 2)
            vec.tensor_scalar(ge[:], xb[:], p256[:], None, op0=ALU.is_ge)
            vec.tensor_scalar(ge2[:], xb[:], p256e[:], None,
                              op0=ALU.is_ge).then_inc(s_v, 1)        # 1
            # fused diff + inclusive scan: state = (gen + state) - ref
            vec.tensor_tensor_scan(scz[:, 1:FP1], frt[:, 0:F],
                                   frt[:, F:2 * F], 0.0,
                                   op0=ALU.add, op1=ALU.subtract)
            vec.tensor_copy(xf[:], frt[:, 3 * F:W].bitcast(I32))
            vec.tensor_scalar(scz_hi[:], scz[:], 0.0, None,
                              op0=ALU.add).then_inc(s_v, 1)          # 2
            vec.tensor_copy(fcf[:], fci[:])
            vec.scalar_tensor_tensor(scz_lo[:], scz[:], 0.0, scz_hi[:],
                                     op0=ALU.add, op1=ALU.subtract) \
               .then_inc(s_v, 1)                                     # 3
            vec.tensor_sub(msum[:], xf[:, 1:2], xf[:, 0:1])
            vector.wait_ge(s_g, 3)     # iota_f
            vec.tensor_scalar(onef_a[:], iota_f[:], fcf[:, 0:1], None,
                              op0=ALU.is_equal)
            vec.tensor_scalar(onef_b[:], iota_f[:], fcf[:, 1:2], None,
                              op0=ALU.is_equal)
            # gather tails: select col c of the gathered row via one-hot
            # multiply + DVE accumulator
            vector.wait_ge(s_t, 1)     # rga
            vec.scalar_tensor_tensor(scr[:], rga_ps[:], 1.0, onef_a[:],
                                     op0=ALU.mult, op1=ALU.mult,
                                     accum_out=val[:, 0:1])
            vector.wait_ge(s_t, 2)     # rgb
            vec.scalar_tensor_tensor(scr[:], rgb_ps[:], 1.0, onef_b[:],
                                     op0=ALU.mult, op1=ALU.mult,
                                     accum_out=val[:, 1:2])
            vec.tensor_scalar(halfm[:], msum[:], 0.5, None, op0=ALU.mult)
            vector.wait_ge(s_t, 3)     # d2
            vec.scalar_tensor_tensor(delta[:], val[:, 1:2], val[:, 0:1],
                                     d2_ps[:], op0=ALU.subtract,
                                     op1=ALU.add)
            vec.tensor_scalar(posb[:], msum[:], 0.0, None, op0=ALU.is_gt)
            vec.scalar_tensor_tensor(absd[:], delta[:], -1.0, delta[:],
                                     op0=ALU.mult, op1=ALU.max)
            vec.tensor_scalar(scr0[:], msum[:], 4.0, None, op0=ALU.mult)
            vec.tensor_tensor(cmpb[:], halfm[:], absd[:],
                              ALU.is_lt).then_inc(s_v, 1)            # 4
            vector.wait_ge(s_t, 4)     # verdict count in PSUM
            vec.tensor_copy(vs_s[0:1, 0:1], vs_ps[0:1, 0:1]) \
               .then_inc(s_v, 1)                                     # 5
            if debug_outs:
                vec.tensor_copy(rga_cp[:], rga_ps[:])
                vec.tensor_copy(rgb_cp[:], rgb_ps[:])
                vec.tensor_copy(d2_cp[:], d2_ps[:]).then_inc(s_v, 1)  # 6

    nc.finalize()
    return nc


_NC_CACHE = {}


def _get_nc(debug_outs=False):
    if debug_outs not in _NC_CACHE:
        _NC_CACHE[debug_outs] = build_nc(debug_outs)
    return _NC_CACHE[debug_outs]


def _pack_onoff(on, off):
    # [pairs (on_p, off_p) x128 | on x128 | off x128]
    pairs = np.stack([on, off], axis=1).ravel()
    return np.concatenate([pairs, on, off])


def _pack_f0x(gen, ref, on, off):
    # per row: [gen | ref | onset++offset ints bitcast to f32, replicated
    # per row | this row's own (onset, offset) pair]
    g = gen.reshape(P, F)
    r = ref.reshape(P, F)
    idx = np.concatenate([on, off]).astype(np.int32).view(np.float32)
    pair = np.stack([on, off], axis=1).astype(np.int32).view(np.float32)
    return np.concatenate(
        [g, r, np.broadcast_to(idx, (P, K2)), pair], axis=1).copy()


def _run(inputs, debug_outs=False, **kwargs):
    gen = np.ascontiguousarray(inputs["gen_f0"], dtype=np.float32)
    ref = np.ascontiguousarray(inputs["t_f0"], dtype=np.float32)
    on = np.ascontiguousarray(inputs["onset_times"], dtype=np.int32)
    off = np.ascontiguousarray(inputs["offset_times"], dtype=np.int32)

    nc = _get_nc(debug_outs)
    in_maps = [
        {
            "f0x": _pack_f0x(gen, ref, on[c * NPC:(c + 1) * NPC],
                             off[c * NPC:(c + 1) * NPC]),
            "onoff": _pack_onoff(on[c * NPC:(c + 1) * NPC],
                                 off[c * NPC:(c + 1) * NPC]),
        }
        for c in range(NCORES)
    ]
    return run_bass_kernel_spmd(nc, in_maps, core_ids=list(range(NCORES)),
                                **kwargs)


def kernel(**inputs):
    res = _run(inputs)
    counts = np.stack([res.results[c]["verdict"] for c in range(NCORES)])
    return np.asarray(counts.sum() / np.float32(N), dtype=np.float32)


# revision 31
# speedup vs baseline: 1.2075x; 1.0476x over previous
"""Trainium2 Bass kernel for nn_PitchLoss (segment_reduce).

Math: for each note k with frame range [a_k, b_k), the reference builds a
dense (T, N) mask and computes per-note means of gen_f0 / t_f0 over the
range, then loss = mean((|mean_gen - mean_ref| > 0.5)).

Since each note is a contiguous frame range, per-note sums are prefix-sum
differences: with d = gen_f0 - t_f0 and cse[x] = sum(d[0:x]),
    |mean_gen_k - mean_ref_k| = |cse[b_k] - cse[a_k]| / (b_k - a_k)
so  verdict_k = (b_k > a_k) & (|cse[b_k] - cse[a_k]| > 0.5 * (b_k - a_k))
which also reproduces the reference's empty-segment NaN > 0.5 == False.

Sharding: notes across 8 cores (128 notes/core); gen_f0/t_f0 replicated.

Per core, with d laid out (128, 256) and scz = per-row inclusive scan
(col 0 = 0, col 256 = row sum), split x = 256r + c (c = x & 255):
    cse[x] = SUM_q [x >= 256(q+1)] * rowsum[q]              (W2 gather)
           + SUM_p [256p <= x < 256p+256] * scz[p, c]       (onep gather)
Both gathers are one-hot matmuls; the onep gather's in-row column select
uses a DVE one-hot multiply + accumulator, and the W2 gather runs with
difference weights w2d = W2_b - W2_a in {-1,0,1} so one matmul yields
d2 = roffsum[b] - roffsum[a] directly.  x == T falls out naturally: the
onep column is all-zero (contributes 0) and W2 sums every row.
delta = (val_b - val_a) + d2; the verdict count is reduced on-device to
ONE scalar via a final pos^T @ cmp matmul; the host sums 8 counts ->
loss (/1024 is a pow2, so the host mean is exact).

Layout: the host packs one (128, 772) fp32 tensor per core, each row =
[gen (256) | ref (256) | the 256 onset/offset times as FLOATS,
replicated per row | this row's own (onset, offset) pair as floats |
this row's own (onset & 255, offset & 255) pair as floats].  Shipping
the same integers in float encoding (and their low-8-bit column
remainder) removes four on-device int->float conversion ops from the
serial DVE chain (~0.7us).  Each DMA queue loads its 64 rows as ONE
DMA with one 3088-byte descriptor per row.  DMA cost here is
descriptor-count-bound (~100ns+ per descriptor aggregate): splitting
the load into per-slice DMAs (6x64x1KB) measured ~2.5us SLOWER than
128x3KB, same-address broadcast reads ran at ~60GB/s, and gpsimd
partition_broadcast pulls in a different ucode library (~8us
MODIFY_POOL_CONFIG load).  Replicating the indices per row costs 128KB
of extra wire (~0.4us) and is the cheapest option.

Perf notes vs the first working version (28.3us):
 - Output is 1 fp32 (one DMA descriptor).  The old (128,1) verdict DMA
   needed 128 4-byte descriptors; their completion increments trickled
   in over ~6us (descriptor processing dominates tiny SBUF-source DMAs).
   (An extra early "prewarm" fire of this DMA was tried and measured
   NET-NEGATIVE: the second dispatch serializes behind it on the queue
   engine, costing more than the ~0.4us ring-wakeup it saves.)
 - All matmuls run bf16 single-pass with an hi/lo split of the fp32 scan
   (hi = bf16(x), lo = bf16(x - hi), accumulated in the same PSUM bank).
   One-hot weights are exact in bf16; |delta| error <= ~1e-3 against a
   >= 0.2 decision margin on this input.  An fp32 matmul costs
   2x(LDWEIGHTS+MATMUL) passes (~2.1us for 128x257); bf16 pairs ~0.7us.
 - hi/lo splits use TENSOR_SCALAR/STT forms (~350-420ns) instead of
   CAST/TENSOR_TENSOR (~610-710ns for 257 cols).
 - ge/ge2 run before the scan so gpsimd's ~1us tensor_tensor combine
   (onep = ge - ge2) overlaps the scan and the hi/lo split.
 - No Activation-engine compute: an ACTIVATE pulls a ~1.3us
   ACT_TABLE_LOAD into the scalar engine's preamble, delaying its DMA
   dispatch.  The final PSUM->SBUF copy runs on DVE instead.
 - Raw Bacc engine programs with hand-placed semaphores (no TileContext -
   its entry/exit barrier costs ~15us on a ~5us kernel).  Engine-order
   hazards: DVE reads racing the immediately preceding op's writeback are
   padded with real ops (interleaved independent chains; memset does NOT
   count); gpsimd ucode cores overlap, so every gpsimd op incs s_g and
   consumers wait on cumulative counts.
"""

from contextlib import ExitStack

import numpy as np

import concourse.bacc as bacc
import concourse.bass as bass
from concourse import mybir
from concourse.bass_utils import run_bass_kernel_spmd

T = 32768           # frames
N = 1024            # notes
NCORES = 8
NPC = N // NCORES   # notes per core
P = 128             # partitions
F = T // P          # 256 frames per partition row
FP1 = F + 1         # 257: scan columns (col 256 = row sum)
K2 = 2 * NPC        # 256: onsets ++ offsets
W = 3 * F + 4       # 772 packed row floats [gen|ref|xf32|pair|fcol pair]
PLO = 64            # row split between the two DMA queues
DT = mybir.dt.float32
BF = mybir.dt.bfloat16
I32 = mybir.dt.int32
ALU = mybir.AluOpType


def build_nc(debug_outs=False):
    # detect_race_conditions=False: the CoreSim race detector does not credit
    # same-engine program order, but HW engines execute their queues in order
    # (DVE drains its pipe after every op); gpsimd, whose ucode cores do
    # overlap, is synchronized explicitly below.
    nc = bacc.Bacc("TRN2", target_bir_lowering=False, debug=False,
                   detect_race_conditions=False)
    f0x = nc.dram_tensor("f0x", [P, W], DT, kind="ExternalInput")
    onoff = nc.dram_tensor("onoff", [2 * K2], I32, kind="ExternalInput")
    out = nc.dram_tensor("verdict", [1], DT, kind="ExternalOutput")
    dbg = {}
    if debug_outs:
        for name, shape, dt in [
                ("dbg_scz", [P, FP1], DT),
                ("dbg_onep", [P, K2], BF), ("dbg_w2d", [P, NPC], BF),
                ("dbg_onefa", [P, F], DT), ("dbg_onefb", [P, F], DT),
                ("dbg_val", [P, 2], DT),
                ("dbg_szhi", [P, FP1], BF), ("dbg_szlo", [P, FP1], BF),
                ("dbg_msum", [P, 1], DT), ("dbg_halfm", [P, 1], DT),
                ("dbg_d2", [P, 1], DT),
                ("dbg_delta", [P, 1], DT), ("dbg_absd", [P, 1], DT),
                ("dbg_cmp", [P, 1], BF), ("dbg_pos", [P, 1], BF),
                ("dbg_rga", [P, F], DT), ("dbg_rgb", [P, F], DT)]:
            dbg[name] = nc.dram_tensor(name, shape, dt, kind="ExternalOutput")

    with ExitStack() as ctx:
        def sb(name, shape, dt=DT):
            return ctx.enter_context(nc.sbuf_tensor(name, shape, dt))

        def pst(name, shape):
            return ctx.enter_context(nc.psum_tensor(name, shape, DT))

        # constants
        p256 = sb("p256", [P, 1])          # 256p
        p256e = sb("p256e", [P, 1])        # 256p + 256
        iota_f = sb("iota_f", [P, F])      # 0..255 per row
        # data tiles
        frt = sb("frt", [P, W])            # [gen|ref|xf32|pair|fcol]
        ge = sb("ge", [P, K2])
        ge2 = sb("ge2", [P, K2])
        onep = sb("onep", [P, K2], BF)
        w2d = sb("w2d", [P, NPC], BF)
        msum = sb("msum", [P, 1])
        halfm = sb("halfm", [P, 1])
        posb = sb("posb", [P, 1], BF)
        scr0 = sb("scr0", [P, 1])          # spacer scratch
        scz = sb("scz", [P, FP1])
        scz_hi = sb("scz_hi", [P, FP1], BF)
        scz_lo = sb("scz_lo", [P, FP1], BF)
        onef_a = sb("onef_a", [P, F])
        onef_b = sb("onef_b", [P, F])
        scr = sb("scr", [P, F])
        val = sb("val", [P, 2])
        delta = sb("delta", [P, 1])
        absd = sb("absd", [P, 1])
        cmpb = sb("cmpb", [P, 1], BF)
        vs_s = sb("vs_s", [1, 1])
        if debug_outs:
            rga_cp = sb("rga_cp", [P, F])
            rgb_cp = sb("rgb_cp", [P, F])
            d2_cp = sb("d2_cp", [P, 1])
        # psum (distinct banks)
        rga_ps = pst("rga_ps", [P, F])
        rgb_ps = pst("rgb_ps", [P, F])
        d2_ps = pst("d2_ps", [P, 1])
        vs_ps = pst("vs_ps", [1, 1])

        s_fr = ctx.enter_context(nc.semaphore("s_fr"))
        s_fr2 = ctx.enter_context(nc.semaphore("s_fr2"))
        s_g = ctx.enter_context(nc.semaphore("s_g"))
        s_v = ctx.enter_context(nc.semaphore("s_v"))
        s_t = ctx.enter_context(nc.semaphore("s_t"))
        s_out = ctx.enter_context(nc.semaphore("s_out"))
        block = ctx.enter_context(nc.Block())

        @block.sync
        def _(sync):
            sync.dma_start(out=frt[0:PLO, :],
                           in_=f0x[0:PLO, :]).then_inc(s_fr, 16)
            sync.wait_ge(s_v, 5)
            sync.dma_start(out=out[:].rearrange("(p f) -> p f", f=1),
                           in_=vs_s[0:1, 0:1]).then_inc(s_out, 16)
            n_out = 16
            if debug_outs:
                sync.wait_ge(s_v, 6)
                sync.wait_ge(s_g, 5)
                for name, tile in [
                        ("dbg_scz", scz),
                        ("dbg_onep", onep), ("dbg_w2d", w2d),
                        ("dbg_onefa", onef_a), ("dbg_onefb", onef_b),
                        ("dbg_val", val),
                        ("dbg_szhi", scz_hi), ("dbg_szlo", scz_lo),
                        ("dbg_msum", msum), ("dbg_halfm", halfm),
                        ("dbg_d2", d2_cp),
                        ("dbg_delta", delta), ("dbg_absd", absd),
                        ("dbg_cmp", cmpb), ("dbg_pos", posb),
                        ("dbg_rga", rga_cp), ("dbg_rgb", rgb_cp)]:
                    sync.dma_start(out=dbg[name][:], in_=tile[:]) \
                        .then_inc(s_out, 16)
                    n_out += 16
            sync.wait_ge(s_out, n_out)

        @block.scalar
        def _(act):
            act.dma_start(out=frt[PLO:P, :],
                          in_=f0x[PLO:P, :]).then_inc(s_fr2, 16)

        @block.gpsimd
        def _(gpsimd):
            # gpsimd ucode cores overlap: every op incs s_g; consumers of a
            # gpsimd result wait on the cumulative count (all earlier-issued
            # ops complete by then, since each op incs exactly once).
            gp = nc.gpsimd
            gpsimd.iota(p256[:], pattern=[[0, 1]], base=0,
                        channel_multiplier=F,
                        allow_small_or_imprecise_dtypes=True).then_inc(s_g, 1)
            gpsimd.iota(p256e[:], pattern=[[0, 1]], base=F,
                        channel_multiplier=F,
                        allow_small_or_imprecise_dtypes=True).then_inc(s_g, 1)
            gpsimd.iota(iota_f[:], pattern=[[1, F]], base=0,
                        channel_multiplier=0,
                        allow_small_or_imprecise_dtypes=True).then_inc(s_g, 1)
            # one-hot matmul weights from the DVE compares:
            # onep[p,k] = [256p <= x_k < 256p+256] = ge - ge2 (0/1, bf16-exact)
            # w2d[p,k]  = [b_k >= 256(p+1)] - [a_k >= 256(p+1)]  in {-1,0,1}
            gpsimd.wait_ge(s_v, 1)     # ge + ge2
            gp.tensor_tensor(onep[:], ge[:], ge2[:],
                             ALU.subtract).then_inc(s_g, 1)          # 4
            gp.tensor_tensor(w2d[:], ge2[:, NPC:K2], ge2[:, 0:NPC],
                             ALU.subtract).then_inc(s_g, 1)          # 5

        @block.tensor
        def _(tensor):
            # all matmuls bf16 single-pass; fp32 scan data enters as hi/lo
            # bf16 pairs accumulated in the same PSUM bank.
            tensor.wait_ge(s_g, 4)     # onep
            tensor.wait_ge(s_v, 2)     # scz_hi
            nc.tensor.matmul(rga_ps[:], onep[:, 0:NPC], scz_hi[:, 0:F],
                             start=True, stop=False)
            tensor.wait_ge(s_v, 3)     # scz_lo
            nc.tensor.matmul(rga_ps[:], onep[:, 0:NPC], scz_lo[:, 0:F],
                             start=False, stop=True).then_inc(s_t, 1)
            nc.tensor.matmul(rgb_ps[:], onep[:, NPC:K2], scz_hi[:, 0:F],
                             start=True, stop=False)
            nc.tensor.matmul(rgb_ps[:], onep[:, NPC:K2], scz_lo[:, 0:F],
                             start=False, stop=True).then_inc(s_t, 1)
            tensor.wait_ge(s_g, 5)     # w2d
            nc.tensor.matmul(d2_ps[:], w2d[:], scz_hi[:, F:FP1],
                             start=True, stop=False)
            nc.tensor.matmul(d2_ps[:], w2d[:], scz_lo[:, F:FP1],
                             start=False, stop=True).then_inc(s_t, 1)
            tensor.wait_ge(s_v, 4)     # cmpb (posb is earlier in v-order)
            nc.tensor.matmul(vs_ps[0:1, 0:1], posb[:], cmpb[:],
                             start=True, stop=True).then_inc(s_t, 1)

        @block.vector
        def _(vector):
            vec = nc.vector
            vec.memset(scz[:, 0:1], 0.0)
            # DVE gap-0 RAW hazards get >=1 real op between each dependent
            # pair (interleaved independent chains; memset does NOT count).
            vector.wait_ge(s_fr, 16)
            vector.wait_ge(s_fr2, 16)
            vector.wait_ge(s_g, 2)
            vec.tensor_scalar(ge[:], frt[:, 2 * F:3 * F], p256[:], None,
                              op0=ALU.is_ge)
            vec.tensor_scalar(ge2[:], frt[:, 2 * F:3 * F], p256e[:], None,
                              op0=ALU.is_ge).then_inc(s_v, 1)        # 1
            # fused diff + inclusive scan: state = (gen + state) - ref
            vec.tensor_tensor_scan(scz[:, 1:FP1], frt[:, 0:F],
                                   frt[:, F:2 * F], 0.0,
                                   op0=ALU.add, op1=ALU.subtract)
            vec.tensor_sub(msum[:], frt[:, 3 * F + 1:3 * F + 2],
                           frt[:, 3 * F:3 * F + 1])
            vec.tensor_scalar(scz_hi[:], scz[:], 0.0, None,
                              op0=ALU.add).then_inc(s_v, 1)          # 2
            vec.tensor_scalar(halfm[:], msum[:], 0.5, None, op0=ALU.mult)
            vec.scalar_tensor_tensor(scz_lo[:], scz[:], 0.0, scz_hi[:],
                                     op0=ALU.add, op1=ALU.subtract) \
               .then_inc(s_v, 1)                                     # 3
            vector.wait_ge(s_g, 3)     # iota_f
            vec.tensor_scalar(onef_a[:], iota_f[:],
                              frt[:, 3 * F + 2:3 * F + 3], None,
                              op0=ALU.is_equal)
            vec.tensor_scalar(onef_b[:], iota_f[:],
                              frt[:, 3 * F + 3:W], None,
                              op0=ALU.is_equal)
            # gather tails: select col c of the gathered row via one-hot
            # multiply + DVE accumulator
            vector.wait_ge(s_t, 1)     # rga
            vec.scalar_tensor_tensor(scr[:], rga_ps[:], 1.0, onef_a[:],
                                     op0=ALU.mult, op1=ALU.mult,
                                     accum_out=val[:, 0:1])
            vector.wait_ge(s_t, 2)     # rgb
            vec.scalar_tensor_tensor(scr[:], rgb_ps[:], 1.0, onef_b[:],
                                     op0=ALU.mult, op1=ALU.mult,
                                     accum_out=val[:, 1:2])
            vec.tensor_scalar(posb[:], msum[:], 0.0, None, op0=ALU.is_gt)
            vector.wait_ge(s_t, 3)     # d2
            vec.scalar_tensor_tensor(delta[:], val[:, 1:2], val[:, 0:1],
                                     d2_ps[:], op0=ALU.subtract,
                                     op1=ALU.add)
            vec.tensor_scalar(scr0[:], msum[:], 3.0, None, op0=ALU.mult)
            vec.scalar_tensor_tensor(absd[:], delta[:], -1.0, delta[:],
                                     op0=ALU.mult, op1=ALU.max)
            vec.tensor_scalar(scr0[:], msum[:], 4.0, None, op0=ALU.mult)
            vec.tensor_tensor(cmpb[:], halfm[:], absd[:],
                              ALU.is_lt).then_inc(s_v, 1)            # 4
            vector.wait_ge(s_t, 4)     # verdict count in PSUM
            vec.tensor_copy(vs_s[0:1, 0:1], vs_ps[0:1, 0:1]) \
               .then_inc(s_v, 1)                                     # 5
            if debug_outs:
                vec.tensor_copy(rga_cp[:], rga_ps[:])
                vec.tensor_copy(rgb_cp[:], rgb_ps[:])
                vec.tensor_copy(d2_cp[:], d2_ps[:]).then_inc(s_v, 1)  # 6

    nc.finalize()
    return nc


_NC_CACHE = {}


def _get_nc(debug_outs=False):
    if debug_outs not in _NC_CACHE:
        _NC_CACHE[debug_outs] = build_nc(debug_outs)
    return _NC_CACHE[debug_outs]


def _pack_onoff(on, off):
    # [pairs (on_p, off_p) x128 | on x128 | off x128]
    pairs = np.stack([on, off], axis=1).ravel()
    return np.concatenate([pairs, on, off])


def _pack_f0x(gen, ref, on, off):
    # per row: [gen | ref | onset++offset times as f32, replicated per row
    # | this row's own (onset, offset) pair as f32 | its (onset & 255,
    # offset & 255) in-row columns as f32] -- same integers, float
    # encoding, so the device needs no int->float conversion ops
    g = gen.reshape(P, F)
    r = ref.reshape(P, F)
    idx = np.concatenate([on, off]).astype(np.float32)
    pair = np.stack([on, off], axis=1).astype(np.float32)
    fcol = np.stack([on & (F - 1), off & (F - 1)],
                    axis=1).astype(np.float32)
    return np.concatenate(
        [g, r, np.broadcast_to(idx, (P, K2)), pair, fcol], axis=1).copy()


def _run(inputs, debug_outs=False, **kwargs):
    gen = np.ascontiguousarray(inputs["gen_f0"], dtype=np.float32)
    ref = np.ascontiguousarray(inputs["t_f0"], dtype=np.float32)
    on = np.ascontiguousarray(inputs["onset_times"], dtype=np.int32)
    off = np.ascontiguousarray(inputs["offset_times"], dtype=np.int32)

    nc = _get_nc(debug_outs)
    in_maps = [
        {
            "f0x": _pack_f0x(gen, ref, on[c * NPC:(c + 1) * NPC],
                             off[c * NPC:(c + 1) * NPC]),
            "onoff": _pack_onoff(on[c * NPC:(c + 1) * NPC],
                                 off[c * NPC:(c + 1) * NPC]),
        }
        for c in range(NCORES)
    ]
    return run_bass_kernel_spmd(nc, in_maps, core_ids=list(range(NCORES)),
                                **kwargs)


def kernel(**inputs):
    res = _run(inputs)
    counts = np.stack([res.results[c]["verdict"] for c in range(NCORES)])
    return np.asarray(counts.sum() / np.float32(N), dtype=np.float32)


# revision 34
# speedup vs baseline: 1.2237x; 1.0134x over previous
"""Trainium2 Bass kernel for nn_PitchLoss (segment_reduce).

Math: for each note k with frame range [a_k, b_k), the reference builds a
dense (T, N) mask and computes per-note means of gen_f0 / t_f0 over the
range, then loss = mean((|mean_gen - mean_ref| > 0.5)).

Since each note is a contiguous frame range, per-note sums are prefix-sum
differences: with d = gen_f0 - t_f0 and cse[x] = sum(d[0:x]),
    |mean_gen_k - mean_ref_k| = |cse[b_k] - cse[a_k]| / (b_k - a_k)
so  verdict_k = (b_k > a_k) & (|cse[b_k] - cse[a_k]| > 0.5 * (b_k - a_k))
which also reproduces the reference's empty-segment NaN > 0.5 == False.

Sharding: notes across 8 cores (128 notes/core); gen_f0/t_f0 replicated.

Per core, with d laid out (128, 256) and scz = per-row inclusive scan
(col 0 = 0, col 256 = row sum), split x = 256r + c (c = x & 255):
    cse[x] = SUM_q [x >= 256(q+1)] * rowsum[q]              (W2 gather)
           + SUM_p [256p <= x < 256p+256] * scz[p, c]       (onep gather)
Both gathers are one-hot matmuls; the onep gather's in-row column select
uses a DVE one-hot multiply + accumulator, and the W2 gather runs with
difference weights w2d = W2_b - W2_a in {-1,0,1} so one matmul yields
d2 = roffsum[b] - roffsum[a] directly.  x == T falls out naturally: the
onep column is all-zero (contributes 0) and W2 sums every row.
delta = (val_b - val_a) + d2; the verdict count is reduced on-device to
ONE scalar via a final pos^T @ cmp matmul; the host sums 8 counts ->
loss (/1024 is a pow2, so the host mean is exact).

Layout: the host packs one (128, 772) fp32 tensor per core, each row =
[gen (256) | ref (256) | the 256 onset/offset times as FLOATS,
replicated per row | this row's own (onset, offset) pair as floats |
this row's own (onset & 255, offset & 255) pair as floats].  Shipping
the same integers in float encoding (and their low-8-bit column
remainder) removes four on-device int->float conversion ops from the
serial DVE chain (~0.7us).  Each DMA queue loads its 64 rows as ONE
DMA with one 3088-byte descriptor per row.  DMA cost here is
descriptor-count-bound (~100ns+ per descriptor aggregate): splitting
the load into per-slice DMAs (6x64x1KB) measured ~2.5us SLOWER than
128x3KB, same-address broadcast reads ran at ~60GB/s, and gpsimd
partition_broadcast pulls in a different ucode library (~8us
MODIFY_POOL_CONFIG load).  Replicating the indices per row costs 128KB
of extra wire (~0.4us) and is the cheapest option.

Perf notes vs the first working version (28.3us):
 - Output is 1 fp32 (one DMA descriptor).  The old (128,1) verdict DMA
   needed 128 4-byte descriptors; their completion increments trickled
   in over ~6us (descriptor processing dominates tiny SBUF-source DMAs).
   (An extra early "prewarm" fire of this DMA was tried and measured
   NET-NEGATIVE: the second dispatch serializes behind it on the queue
   engine, costing more than the ~0.4us ring-wakeup it saves.)
 - All matmuls run bf16 single-pass with an hi/lo split of the fp32 scan
   (hi = bf16(x), lo = bf16(x - hi), accumulated in the same PSUM bank).
   One-hot weights are exact in bf16; |delta| error <= ~1e-3 against a
   >= 0.2 decision margin on this input.  An fp32 matmul costs
   2x(LDWEIGHTS+MATMUL) passes (~2.1us for 128x257); bf16 pairs ~0.7us.
 - hi/lo splits use TENSOR_SCALAR/STT forms (~350-420ns) instead of
   CAST/TENSOR_TENSOR (~610-710ns for 257 cols).
 - ge/ge2 run before the scan so gpsimd's ~1us tensor_tensor combine
   (onep = ge - ge2) overlaps the scan and the hi/lo split.
 - No Activation-engine compute: an ACTIVATE pulls a ~1.3us
   ACT_TABLE_LOAD into the scalar engine's preamble, delaying its DMA
   dispatch.  The final PSUM->SBUF copy runs on DVE instead.
 - Raw Bacc engine programs with hand-placed semaphores (no TileContext -
   its entry/exit barrier costs ~15us on a ~5us kernel).  Engine-order
   hazards: DVE reads racing the immediately preceding op's writeback are
   padded with real ops (interleaved independent chains; memset does NOT
   count); gpsimd ucode cores overlap, so every gpsimd op incs s_g and
   consumers wait on cumulative counts.
"""

from contextlib import ExitStack

import numpy as np

import concourse.bacc as bacc
import concourse.bass as bass
from concourse import mybir
from concourse.bass_utils import run_bass_kernel_spmd

T = 32768           # frames
N = 1024            # notes
NCORES = 8
NPC = N // NCORES   # notes per core
P = 128             # partitions
F = T // P          # 256 frames per partition row
FP1 = F + 1         # 257: scan columns (col 256 = row sum)
K2 = 2 * NPC        # 256: onsets ++ offsets
W = 3 * F + 4       # 772 packed row floats [gen|ref|xf32|pair|fcol pair]
PLO = 64            # row split between the two DMA queues
DT = mybir.dt.float32
BF = mybir.dt.bfloat16
I32 = mybir.dt.int32
ALU = mybir.AluOpType


def build_nc(debug_outs=False):
    # detect_race_conditions=False: the CoreSim race detector does not credit
    # same-engine program order, but HW engines execute their queues in order
    # (DVE drains its pipe after every op); gpsimd, whose ucode cores do
    # overlap, is synchronized explicitly below.
    nc = bacc.Bacc("TRN2", target_bir_lowering=False, debug=False,
                   detect_race_conditions=False)
    f0x = nc.dram_tensor("f0x", [P, W], DT, kind="ExternalInput")
    onoff = nc.dram_tensor("onoff", [2 * K2], I32, kind="ExternalInput")
    out = nc.dram_tensor("verdict", [1], DT, kind="ExternalOutput")
    dbg = {}
    if debug_outs:
        for name, shape, dt in [
                ("dbg_scz", [P, FP1], DT),
                ("dbg_onep", [P, K2], BF), ("dbg_w2d", [P, NPC], BF),
                ("dbg_onefa", [P, F], DT), ("dbg_onefb", [P, F], DT),
                ("dbg_val", [P, 2], DT),
                ("dbg_szhi", [P, FP1], BF), ("dbg_szlo", [P, FP1], BF),
                ("dbg_msum", [P, 1], DT), ("dbg_halfm", [P, 1], DT),
                ("dbg_d2", [P, 1], DT),
                ("dbg_delta", [P, 1], DT), ("dbg_absd", [P, 1], DT),
                ("dbg_cmp", [P, 1], BF), ("dbg_pos", [P, 1], BF),
                ("dbg_rga", [P, F], DT), ("dbg_rgb", [P, F], DT)]:
            dbg[name] = nc.dram_tensor(name, shape, dt, kind="ExternalOutput")

    with ExitStack() as ctx:
        def sb(name, shape, dt=DT):
            return ctx.enter_context(nc.sbuf_tensor(name, shape, dt))

        def pst(name, shape):
            return ctx.enter_context(nc.psum_tensor(name, shape, DT))

        # constants
        p256 = sb("p256", [P, 1])          # 256p
        p256e = sb("p256e", [P, 1])        # 256p + 256
        iota_f = sb("iota_f", [P, F])      # 0..255 per row
        # data tiles
        frt = sb("frt", [P, W])            # [gen|ref|xf32|pair|fcol]
        ge = sb("ge", [P, K2])
        ge2 = sb("ge2", [P, K2])
        onep = sb("onep", [P, K2], BF)
        w2d = sb("w2d", [P, NPC], BF)
        msum = sb("msum", [P, 1])
        halfm = sb("halfm", [P, 1])
        posb = sb("posb", [P, 1], BF)
        scr0 = sb("scr0", [P, 1])          # spacer scratch
        scz = sb("scz", [P, FP1])
        scz_hi = sb("scz_hi", [P, FP1], BF)
        scz_lo = sb("scz_lo", [P, FP1], BF)
        onef_a = sb("onef_a", [P, F])
        onef_b = sb("onef_b", [P, F])
        scr = sb("scr", [P, F])
        val = sb("val", [P, 2])
        delta = sb("delta", [P, 1])
        absd = sb("absd", [P, 1])
        cmpb = sb("cmpb", [P, 1], BF)
        vs_s = sb("vs_s", [1, 1])
        if debug_outs:
            rga_cp = sb("rga_cp", [P, F])
            rgb_cp = sb("rgb_cp", [P, F])
            d2_cp = sb("d2_cp", [P, 1])
        # psum (distinct banks)
        rga_ps = pst("rga_ps", [P, F])
        rgb_ps = pst("rgb_ps", [P, F])
        d2_ps = pst("d2_ps", [P, 1])
        vs_ps = pst("vs_ps", [1, 1])

        s_fr = ctx.enter_context(nc.semaphore("s_fr"))
        s_fr2 = ctx.enter_context(nc.semaphore("s_fr2"))
        s_g = ctx.enter_context(nc.semaphore("s_g"))
        s_v = ctx.enter_context(nc.semaphore("s_v"))
        s_t = ctx.enter_context(nc.semaphore("s_t"))
        s_out = ctx.enter_context(nc.semaphore("s_out"))
        block = ctx.enter_context(nc.Block())

        @block.sync
        def _(sync):
            sync.dma_start(out=frt[0:PLO, :],
                           in_=f0x[0:PLO, :]).then_inc(s_fr, 16)
            sync.wait_ge(s_v, 5)
            sync.dma_start(out=out[:].rearrange("(p f) -> p f", f=1),
                           in_=vs_s[0:1, 0:1]).then_inc(s_out, 16)
            n_out = 16
            if debug_outs:
                sync.wait_ge(s_v, 6)
                sync.wait_ge(s_g, 6)
                for name, tile in [
                        ("dbg_scz", scz),
                        ("dbg_onep", onep), ("dbg_w2d", w2d),
                        ("dbg_onefa", onef_a), ("dbg_onefb", onef_b),
                        ("dbg_val", val),
                        ("dbg_szhi", scz_hi), ("dbg_szlo", scz_lo),
                        ("dbg_msum", msum), ("dbg_halfm", halfm),
                        ("dbg_d2", d2_cp),
                        ("dbg_delta", delta), ("dbg_absd", absd),
                        ("dbg_cmp", cmpb), ("dbg_pos", posb),
                        ("dbg_rga", rga_cp), ("dbg_rgb", rgb_cp)]:
                    sync.dma_start(out=dbg[name][:], in_=tile[:]) \
                        .then_inc(s_out, 16)
                    n_out += 16
            sync.wait_ge(s_out, n_out)

        @block.scalar
        def _(act):
            act.dma_start(out=frt[PLO:P, :],
                          in_=f0x[PLO:P, :]).then_inc(s_fr2, 16)

        @block.gpsimd
        def _(gpsimd):
            # gpsimd ucode cores overlap: every op incs s_g; consumers of a
            # gpsimd result wait on the cumulative count (all earlier-issued
            # ops complete by then, since each op incs exactly once).
            gp = nc.gpsimd
            gpsimd.iota(p256[:], pattern=[[0, 1]], base=0,
                        channel_multiplier=F,
                        allow_small_or_imprecise_dtypes=True).then_inc(s_g, 1)
            gpsimd.iota(p256e[:], pattern=[[0, 1]], base=F,
                        channel_multiplier=F,
                        allow_small_or_imprecise_dtypes=True).then_inc(s_g, 1)
            gpsimd.iota(iota_f[:], pattern=[[1, F]], base=0,
                        channel_multiplier=0,
                        allow_small_or_imprecise_dtypes=True).then_inc(s_g, 1)
            # one-hot matmul weights from the DVE compares:
            # onep[p,k] = [256p <= x_k < 256p+256] = ge - ge2 (0/1, bf16-exact)
            # w2d[p,k]  = [b_k >= 256(p+1)] - [a_k >= 256(p+1)]  in {-1,0,1}
            # onep is built in onset/offset halves so the rga weight gate
            # clears ~0.5us before the full-width combine would.
            gpsimd.wait_ge(s_v, 1)     # ge + ge2
            gp.tensor_tensor(onep[:, 0:NPC], ge[:, 0:NPC], ge2[:, 0:NPC],
                             ALU.subtract).then_inc(s_g, 1)          # 4
            gp.tensor_tensor(onep[:, NPC:K2], ge[:, NPC:K2],
                             ge2[:, NPC:K2],
                             ALU.subtract).then_inc(s_g, 1)          # 5
            gp.tensor_tensor(w2d[:], ge2[:, NPC:K2], ge2[:, 0:NPC],
                             ALU.subtract).then_inc(s_g, 1)          # 6

        @block.tensor
        def _(tensor):
            # all matmuls bf16 single-pass; fp32 scan data enters as hi/lo
            # bf16 pairs accumulated in the same PSUM bank.  The two hi
            # passes run back-to-back inside the wait for scz_lo (the rgb-hi
            # weight reload is free there), so rgb completes ~0.3us sooner
            # and unblocks the second gather tail earlier.
            tensor.wait_ge(s_g, 4)     # onep onset half
            tensor.wait_ge(s_v, 2)     # scz_hi
            nc.tensor.matmul(rga_ps[:], onep[:, 0:NPC], scz_hi[:, 0:F],
                             start=True, stop=False, skip_group_check=True)
            tensor.wait_ge(s_g, 5)     # onep offset half
            nc.tensor.matmul(rgb_ps[:], onep[:, NPC:K2], scz_hi[:, 0:F],
                             start=True, stop=False, skip_group_check=True)
            tensor.wait_ge(s_v, 3)     # scz_lo
            nc.tensor.matmul(rga_ps[:], onep[:, 0:NPC], scz_lo[:, 0:F],
                             start=False, stop=True,
                             skip_group_check=True).then_inc(s_t, 1)
            nc.tensor.matmul(rgb_ps[:], onep[:, NPC:K2], scz_lo[:, 0:F],
                             start=False, stop=True,
                             skip_group_check=True).then_inc(s_t, 1)
            tensor.wait_ge(s_g, 6)     # w2d
            nc.tensor.matmul(d2_ps[:], w2d[:], scz_hi[:, F:FP1],
                             start=True, stop=False, skip_group_check=True)
            nc.tensor.matmul(d2_ps[:], w2d[:], scz_lo[:, F:FP1],
                             start=False, stop=True,
                             skip_group_check=True).then_inc(s_t, 1)
            tensor.wait_ge(s_v, 4)     # cmpb (posb is earlier in v-order)
            nc.tensor.matmul(vs_ps[0:1, 0:1], posb[:], cmpb[:],
                             start=True, stop=True).then_inc(s_t, 1)

        @block.vector
        def _(vector):
            vec = nc.vector
            vec.memset(scz[:, 0:1], 0.0)
            # DVE gap-0 RAW hazards get >=1 real op between each dependent
            # pair (interleaved independent chains; memset does NOT count).
            vector.wait_ge(s_fr, 16)
            vector.wait_ge(s_fr2, 16)
            vector.wait_ge(s_g, 2)
            vec.tensor_scalar(ge[:], frt[:, 2 * F:3 * F], p256[:], None,
                              op0=ALU.is_ge)
            vec.tensor_scalar(ge2[:], frt[:, 2 * F:3 * F], p256e[:], None,
                              op0=ALU.is_ge).then_inc(s_v, 1)        # 1
            # fused diff + inclusive scan: state = (gen + state) - ref
            vec.tensor_tensor_scan(scz[:, 1:FP1], frt[:, 0:F],
                                   frt[:, F:2 * F], 0.0,
                                   op0=ALU.add, op1=ALU.subtract)
            vec.tensor_sub(msum[:], frt[:, 3 * F + 1:3 * F + 2],
                           frt[:, 3 * F:3 * F + 1])
            vec.tensor_scalar(scz_hi[:], scz[:], 0.0, None,
                              op0=ALU.add).then_inc(s_v, 1)          # 2
            vec.tensor_scalar(halfm[:], msum[:], 0.5, None, op0=ALU.mult)
            vec.scalar_tensor_tensor(scz_lo[:], scz[:], 0.0, scz_hi[:],
                                     op0=ALU.add, op1=ALU.subtract) \
               .then_inc(s_v, 1)                                     # 3
            vector.wait_ge(s_g, 3)     # iota_f
            vec.tensor_scalar(onef_a[:], iota_f[:],
                              frt[:, 3 * F + 2:3 * F + 3], None,
                              op0=ALU.is_equal)
            vec.tensor_scalar(onef_b[:], iota_f[:],
                              frt[:, 3 * F + 3:W], None,
                              op0=ALU.is_equal)
            # gather tails: select col c of the gathered row via one-hot
            # multiply + DVE accumulator
            vector.wait_ge(s_t, 1)     # rga
            vec.scalar_tensor_tensor(scr[:], rga_ps[:], 1.0, onef_a[:],
                                     op0=ALU.mult, op1=ALU.mult,
                                     accum_out=val[:, 0:1])
            vector.wait_ge(s_t, 2)     # rgb
            vec.scalar_tensor_tensor(scr[:], rgb_ps[:], 1.0, onef_b[:],
                                     op0=ALU.mult, op1=ALU.mult,
                                     accum_out=val[:, 1:2])
            vec.tensor_scalar(posb[:], msum[:], 0.0, None, op0=ALU.is_gt)
            vector.wait_ge(s_t, 3)     # d2
            vec.scalar_tensor_tensor(delta[:], val[:, 1:2], val[:, 0:1],
                                     d2_ps[:], op0=ALU.subtract,
                                     op1=ALU.add)
            vec.tensor_scalar(scr0[:], msum[:], 3.0, None, op0=ALU.mult)
            vec.scalar_tensor_tensor(absd[:], delta[:], -1.0, delta[:],
                                     op0=ALU.mult, op1=ALU.max)
            vec.tensor_scalar(scr0[:], msum[:], 4.0, None, op0=ALU.mult)
            vec.tensor_tensor(cmpb[:], halfm[:], absd[:],
                              ALU.is_lt).then_inc(s_v, 1)            # 4
            vector.wait_ge(s_t, 4)     # verdict count in PSUM
            vec.tensor_copy(vs_s[0:1, 0:1], vs_ps[0:1, 0:1]) \
               .then_inc(s_v, 1)                                     # 5
            if debug_outs:
                vec.tensor_copy(rga_cp[:], rga_ps[:])
                vec.tensor_copy(rgb_cp[:], rgb_ps[:])
                vec.tensor_copy(d2_cp[:], d2_ps[:]).then_inc(s_v, 1)  # 6

    nc.finalize()
    return nc


_NC_CACHE = {}


def _get_nc(debug_outs=False):
    if debug_outs not in _NC_CACHE:
        _NC_CACHE[debug_outs] = build_nc(debug_outs)
    return _NC_CACHE[debug_outs]


def _pack_onoff(on, off):
    # [pairs (on_p, off_p) x128 | on x128 | off x128]
    pairs = np.stack([on, off], axis=1).ravel()
    return np.concatenate([pairs, on, off])


def _pack_f0x(gen, ref, on, off):
    # per row: [gen | ref | onset++offset times as f32, replicated per row
    # | this row's own (onset, offset) pair as f32 | its (onset & 255,
    # offset & 255) in-row columns as f32] -- same integers, float
    # encoding, so the device needs no int->float conversion ops
    g = gen.reshape(P, F)
    r = ref.reshape(P, F)
    idx = np.concatenate([on, off]).astype(np.float32)
    pair = np.stack([on, off], axis=1).astype(np.float32)
    fcol = np.stack([on & (F - 1), off & (F - 1)],
                    axis=1).astype(np.float32)
    return np.concatenate(
        [g, r, np.broadcast_to(idx, (P, K2)), pair, fcol], axis=1).copy()


def _run(inputs, debug_outs=False, **kwargs):
    gen = np.ascontiguousarray(inputs["gen_f0"], dtype=np.float32)
    ref = np.ascontiguousarray(inputs["t_f0"], dtype=np.float32)
    on = np.ascontiguousarray(inputs["onset_times"], dtype=np.int32)
    off = np.ascontiguousarray(inputs["offset_times"], dtype=np.int32)

    nc = _get_nc(debug_outs)
    in_maps = [
        {
            "f0x": _pack_f0x(gen, ref, on[c * NPC:(c + 1) * NPC],
                             off[c * NPC:(c + 1) * NPC]),
            "onoff": _pack_onoff(on[c * NPC:(c + 1) * NPC],
                                 off[c * NPC:(c + 1) * NPC]),
        }
        for c in range(NCORES)
    ]
    return run_bass_kernel_spmd(nc, in_maps, core_ids=list(range(NCORES)),
                                **kwargs)


def kernel(**inputs):
    res = _run(inputs)
    counts = np.stack([res.results[c]["verdict"] for c in range(NCORES)])
    return np.asarray(counts.sum() / np.float32(N), dtype=np.float32)
